# revision 59
# baseline (speedup 1.0000x reference)
"""DLRM forward (embedding gather + tiny MLPs) as a Bass/Tile kernel on 8 trn2 cores.

Sharding: data-parallel over the batch; each of the 8 cores handles B/8 = 2048
samples end-to-end against a full replica of the (read-only) tables.

Key transformation (host-side, exact): the top-MLP first layer is linear in the
embedding concat, so fold tw1 into the tables once per call:
    PT[f] = tables[f] @ tw1[f*64:(f+1)*64, :]        # [CARD, 16] per table
    hidden[b] = sum_f PT[f][idx[b,f]] + relu(x@w1+b1) @ (w2@tw1_d) + (b2@tw1_d + tb1)
The constant vector (b2@tw1_d + tb1) is folded into PT table 0. The device
then gathers 64B fp32 rows (26 per sample) and reduces them on DVE — no
[B, 26, 64] materialization, no PE transposes of embeddings, and the gather
descriptor stream (53248 64B rows/core) runs at the DMA floor.

Per-core pipeline:
  - 16 indirect DMAs (one per 128-sample tile) gather 26x16 f32 into one big
    SBUF tile; DVE reduces each [128, 26, 16] block over f (axis-X reduce on a
    strided view) into gathersum [128, 16].
  - bottom MLP runs feature-major on PE ([13,2048] loaded pre-transposed from
    host): w1 matmul -> relu -> (w2@tw1_d) matmul -> PE-transpose back to
    sample-major [128, 16] chunks in PSUM.
  - DVE adds gathersum + dense chunk, applies relu*tw2 in one
    scalar_tensor_tensor, reduces over the 16 hidden units; ACT applies
    sigmoid(+tb2); one final PE transpose lays y out [16, 128] for a single
    contiguous output DMA.
"""

import numpy as np

import concourse.bass as bass
import concourse.mybir as mybir
import concourse.tile as tile
from concourse import bacc

P = 128

# Problem constants (hardcoded per harness contract).
N_CORES = 8
B = 16384
F = 26
D = 64
DENSE = 13
CARD = 100000
H_BOT = 8
H_TOP = 16

f32 = mybir.dt.float32
i32 = mybir.dt.int32

GATHER_SIZES = [2, 4, 4, 2, 2, 2]  # tiles of 128 samples per indirect DMA
POOL_GROUP = 3  # group of 4 tiles whose 26-sum + head runs on gpsimd
POOL_FROM = 12  # first tile whose 26-sum runs on gpsimd (rest go to DVE)
TABLE_DT = mybir.dt.float16
TABLE_NP_DT = np.float16
f16 = mybir.dt.float16


def build_kernel(
    b_loc=B // N_CORES,
    card=CARD,
    n_f=F,
    n_dense=DENSE,
    h_bot=H_BOT,
    h_top=H_TOP,
):
    v = n_f * card
    n_t = b_loc // P  # 16 tiles of 128 samples
    group = min(512, b_loc)  # batch columns per matmul group
    tpg = group // P  # tiles per group (4)
    n_g = b_loc // group  # groups (4)
    row = h_top  # gathered row length (16 f32)
    trow = n_f * row  # per-tile gather width (416)

    nc = bacc.Bacc("TRN2", target_bir_lowering=False)
    ptab_d = nc.dram_tensor("ptab", [v, row], TABLE_DT, kind="ExternalInput")
    comb_d = nc.dram_tensor("comb", [P, n_t * n_f], i32, kind="ExternalInput")
    dxt_d = nc.dram_tensor("dxt", [n_dense, b_loc], f16, kind="ExternalInput")
    # wblob[:, 0:8] = w1 (13 rows); wblob[0:8, 8:24] = w2@tw1_d  (fp16)
    wb_d = nc.dram_tensor("wblob", [n_dense, h_bot + h_top], f16, kind="ExternalInput")
    # cblob[:, 0:64] = tw2 tiled 4x; cblob[:, 64] = tb2; cblob[0:8, 65] = b1
    cb_d = nc.dram_tensor("cblob", [P, tpg * h_top + 2], f32, kind="ExternalInput")
    id_d = nc.dram_tensor("identh", [P, P], f16, kind="ExternalInput")
    y_d = nc.dram_tensor("y", [n_t, P], f32, kind="ExternalOutput")

    with tile.TileContext(nc) as tc:
        with (
            tc.tile_pool(name="const", bufs=1) as cpool,
            tc.tile_pool(name="small", bufs=2) as smallp,
            tc.tile_pool(name="pmm", bufs=2, space="PSUM") as pmmp,
            tc.tile_pool(name="pd", bufs=2, space="PSUM") as pdp,
            tc.tile_pool(name="pfix", bufs=1, space="PSUM") as pfixp,
        ):
            # ---- dummy sigmoid: forces the one ACT table set that contains
            # sigmoid+relu+copy to load now, so no reload sits on the tail ----
            scr0 = cpool.tile([1, 1], f32)
            nc.gpsimd.memset(scr0[:], 0.0)
            scr1 = cpool.tile([1, 1], f32)
            nc.scalar.activation(
                out=scr1[:],
                in_=scr0[:],
                func=mybir.ActivationFunctionType.Sigmoid,
            )

            # ---- index upload: sole occupant of the sync (SP) queue, so the
            # first gather waits on exactly one DMA; split so the first
            # gather's slice lands first ----
            c0 = GATHER_SIZES[0] * n_f
            comb = cpool.tile([P, n_t * n_f], i32)
            nc.sync.dma_start(out=comb[:, 0:c0], in_=comb_d[:, 0:c0])
            nc.sync.dma_start(out=comb[:, c0 : n_t * n_f], in_=comb_d[:, c0 : n_t * n_f])

            # ---- constants via the Activation HWDGE queue (independent) ----
            dxt = cpool.tile([n_dense, b_loc], f16)
            nc.scalar.dma_start(out=dxt[:], in_=dxt_d[:, :])
            wb = cpool.tile([n_dense, h_bot + h_top], f16)
            nc.scalar.dma_start(out=wb[:], in_=wb_d[:, :])
            w1_sb = wb[:, 0:h_bot]
            w21_sb = wb[0:h_bot, h_bot : h_bot + h_top]
            cb = cpool.tile([P, tpg * h_top + 2], f32)
            nc.scalar.dma_start(out=cb[:], in_=cb_d[:, :])
            tw2b = cb[:, 0 : tpg * h_top]
            tb2b = cb[:, tpg * h_top : tpg * h_top + 1]
            b1_sb = cb[0:h_bot, tpg * h_top + 1 : tpg * h_top + 2]
            ident = cpool.tile([P, P], f16)
            nc.scalar.dma_start(out=ident[:], in_=id_d[:, :])

            # one SBUF tile per gather chunk: a single big tile would make
            # every reader wait on ALL gathers (subtile dep tracking lumps
            # the regions), serializing the whole reduce pipeline
            et_tiles = {}
            t0 = 0
            for sz in GATHER_SIZES:
                et = cpool.tile([P, sz * trow], TABLE_DT, name=f"et_{t0}")
                for t in range(t0, t0 + sz):
                    et_tiles[t] = (et, t - t0)
                t0 += sz
            gs = cpool.tile([P, n_t * h_top], f32)  # gathersum [128, 256]
            hs = cpool.tile([P, n_t * h_top], f32)  # hidden pre-relu
            mm = cpool.tile([P, n_t * h_top], f32)  # relu(h) * tw2
            lg = cpool.tile([P, n_t], f32)  # logits [128, 16]
            ylog = cpool.tile([P, n_t], f16)
            pdhT = pfixp.tile([P, n_t * h_top], f16)  # dense hidden, sample-major
            # the gpsimd-owned group gets its own PSUM tile: sharing one tile
            # with the DVE heads makes the ACT staging copy inherit their
            # (slice-unrelated) deps and stalls it by ~8us
            pdhT2 = pfixp.tile([P, tpg * h_top], f16)

            # ---- bottom MLP, feature-major, wave-scheduled so the in-order
            # PE queue never stalls on a not-yet-ready cross-engine dep ----
            phs, h1s, pds, dhs = [], [], [], []
            for g in range(n_g):
                ph = pmmp.tile([h_bot, group], f32, tag="pmm")
                nc.tensor.matmul(
                    out=ph[:],
                    lhsT=w1_sb,
                    rhs=dxt[:, bass.ts(g, group)],
                    start=True,
                    stop=True,
                )
                phs.append(ph)
            for g in range(n_g):
                h1 = smallp.tile([h_bot, group], f16, tag="h1")
                nc.scalar.activation(
                    out=h1[:],
                    in_=phs[g][:],
                    func=mybir.ActivationFunctionType.Relu,
                    bias=b1_sb,
                )
                h1s.append(h1)
            for g in range(n_g):
                pd = pdp.tile([h_top, group], f32, tag="pd")
                nc.tensor.matmul(
                    out=pd[:], lhsT=w21_sb, rhs=h1s[g][:], start=True, stop=True
                )
                pds.append(pd)
            for g in range(n_g):
                dh = smallp.tile([h_top, group], f16, tag="dh")
                nc.scalar.activation(
                    out=dh[:],
                    in_=pds[g][:],
                    func=mybir.ActivationFunctionType.Copy,
                )
                dhs.append(dh)
            for g in range(n_g):
                for j in range(tpg):
                    dst = (
                        pdhT2[:, bass.ts(j, h_top)]
                        if g == POOL_GROUP
                        else pdhT[:, bass.ts(g * tpg + j, h_top)]
                    )
                    nc.tensor.transpose(
                        out=dst,
                        in_=dhs[g][:, bass.ts(j, P)],
                        identity=ident[0:h_top, 0:h_top],
                    )

            # ---- gathers, one strided reduce chasing each gather,
            # per-group head ops interleaved ----
            def emit_head(g):
                gcols = bass.ts(g, tpg * h_top)
                nc.vector.tensor_tensor(
                    out=hs[:, gcols],
                    in0=gs[:, gcols],
                    in1=pdhT[:, gcols],
                    op=mybir.AluOpType.add,
                )
                # relu then scale by tw2 (broadcast across partitions)
                nc.vector.scalar_tensor_tensor(
                    out=mm[:, gcols],
                    in0=hs[:, gcols],
                    scalar=0.0,
                    in1=tw2b,
                    op0=mybir.AluOpType.max,
                    op1=mybir.AluOpType.mult,
                )
                nc.vector.tensor_reduce(
                    out=lg[:, bass.ts(g, tpg)],
                    in_=mm[:, gcols].rearrange("p (t j) -> p t j", t=tpg),
                    axis=mybir.AxisListType.X,
                    op=mybir.AluOpType.add,
                )

            p_lo, p_hi = POOL_GROUP * tpg, (POOL_GROUP + 1) * tpg
            t0 = 0
            for gi, sz in enumerate(GATHER_SIZES):
                t1 = min(t0 + sz, n_t)
                et = et_tiles[t0][0]
                nc.gpsimd.indirect_dma_start(
                    out=et[:],
                    out_offset=None,
                    in_=ptab_d[:, :],
                    in_offset=bass.IndirectOffsetOnAxis(
                        ap=comb[:, t0 * n_f : t1 * n_f], axis=0
                    ),
                )
                # sum over the 26 tables, one DVE reduce per 128-sample tile
                for t in range(t0, t1):
                    if t >= POOL_FROM:
                        continue  # gpsimd-owned
                    nc.vector.tensor_reduce(
                        out=gs[:, bass.ts(t, h_top)],
                        in_=et[:, bass.ts(t - t0, trow)].rearrange(
                            "p (f j) -> p j f", f=n_f
                        ),
                        axis=mybir.AxisListType.X,
                        op=mybir.AluOpType.add,
                    )
                    if t % tpg == tpg - 1:
                        emit_head(t // tpg)
                t0 = t1

            # ---- gpsimd-owned group: 26-sum tree + head add on gpsimd (idle
            # after desc gen), in parallel with DVE's tiles ----
            if POOL_GROUP < n_g:
                t = POOL_FROM
                ci = 0
                while t < n_t:
                    et, off = et_tiles[t]
                    assert off == 0, "pool chunks must align to gather tiles"
                    K = et.shape[1] // trow
                    assert t + K <= n_t
                    va = et[:].rearrange("p (t f j) -> p t f j", t=K, f=n_f)
                    scrA = cpool.tile([P, K * 13 * h_top], f16, name=f"scrA{ci}")
                    vA = scrA[:].rearrange("p (t f j) -> p t f j", t=K, f=13)
                    nc.gpsimd.tensor_tensor(
                        out=vA, in0=va[:, :, 0:13], in1=va[:, :, 13:26],
                        op=mybir.AluOpType.add,
                    )
                    scrB = cpool.tile([P, K * 6 * h_top], f16, name=f"scrB{ci}")
                    vB = scrB[:].rearrange("p (t f j) -> p t f j", t=K, f=6)
                    nc.gpsimd.tensor_tensor(
                        out=vB, in0=vA[:, :, 0:6], in1=vA[:, :, 6:12],
                        op=mybir.AluOpType.add,
                    )
                    scrC = cpool.tile([P, K * 3 * h_top], f16, name=f"scrC{ci}")
                    vC = scrC[:].rearrange("p (t f j) -> p t f j", t=K, f=3)
                    nc.gpsimd.tensor_tensor(
                        out=vC, in0=vB[:, :, 0:3], in1=vB[:, :, 3:6],
                        op=mybir.AluOpType.add,
                    )
                    scrD = cpool.tile([P, K * h_top], f16, name=f"scrD{ci}")
                    vD = scrD[:].rearrange("p (t c j) -> p t c j", t=K, c=1)
                    nc.gpsimd.tensor_tensor(
                        out=vD, in0=vC[:, :, 0:1], in1=vC[:, :, 1:2],
                        op=mybir.AluOpType.add,
                    )
                    nc.gpsimd.tensor_tensor(
                        out=vD, in0=vD, in1=vC[:, :, 2:3], op=mybir.AluOpType.add
                    )
                    gsl = gs[:, t * h_top : (t + K) * h_top].rearrange(
                        "p (t c j) -> p t c j", t=K, c=1
                    )
                    nc.gpsimd.tensor_tensor(
                        out=gsl, in0=vD, in1=vA[:, :, 12:13], op=mybir.AluOpType.add
                    )
                    t += K
                    ci += 1
                # head for the gpsimd-owned group;
                # gpsimd cannot read PSUM, so stage dense-hidden via ACT
                dhT_sb = cpool.tile([P, tpg * h_top], f32)
                nc.scalar.activation(
                    out=dhT_sb[:],
                    in_=pdhT2[:],
                    func=mybir.ActivationFunctionType.Copy,
                )
                hsl = hs[:, p_lo * h_top : p_hi * h_top]
                nc.gpsimd.tensor_tensor(
                    out=hsl,
                    in0=gs[:, p_lo * h_top : p_hi * h_top],
                    in1=dhT_sb[:],
                    op=mybir.AluOpType.add,
                )
                # heads for groups whose gs is (partly) gpsimd-written but
                # that read pdhT from PSUM: run on DVE at the end of its queue
                for g_mid in range(POOL_FROM // tpg, POOL_GROUP):
                    emit_head(g_mid)
                # relu*tw2 + reduce for these tiles runs on DVE (appended
                # after its own queue; TensorScalarPtr is not a Pool opcode)
                g_last = POOL_GROUP
                gcols = bass.ts(g_last, tpg * h_top)
                nc.vector.scalar_tensor_tensor(
                    out=mm[:, gcols],
                    in0=hs[:, gcols],
                    scalar=0.0,
                    in1=tw2b,
                    op0=mybir.AluOpType.max,
                    op1=mybir.AluOpType.mult,
                )
                nc.vector.tensor_reduce(
                    out=lg[:, bass.ts(g_last, tpg)],
                    in_=mm[:, gcols].rearrange("p (t j) -> p t j", t=tpg),
                    axis=mybir.AxisListType.X,
                    op=mybir.AluOpType.add,
                )

            nc.scalar.activation(
                out=ylog[:],
                in_=lg[:],
                func=mybir.ActivationFunctionType.Sigmoid,
                bias=tb2b,
            )
            pyT = pfixp.tile([n_t, P], f16)
            nc.tensor.transpose(out=pyT[:], in_=ylog[:], identity=ident[:])
            yT = cpool.tile([n_t, P], f32)
            nc.vector.tensor_copy(out=yT[:], in_=pyT[:])
            nc.sync.dma_start(out=y_d[:, :], in_=yT[:])

    nc.compile()
    return nc


_NC_CACHE = {}


def _get_nc():
    if "nc" not in _NC_CACHE:
        _NC_CACHE["nc"] = build_kernel()
    return _NC_CACHE["nc"]


def make_in_maps(dense_x, sparse_x, tables, w1, b1, w2, b2, tw1, tb1, tw2, tb2):
    tables = np.asarray(tables, dtype=np.float32)
    tw1 = np.asarray(tw1, dtype=np.float32)
    tw2 = np.asarray(tw2, dtype=np.float32)
    w2 = np.asarray(w2, dtype=np.float32)
    b2 = np.asarray(b2, dtype=np.float32)
    tb1 = np.asarray(tb1, dtype=np.float32)
    tb2 = np.asarray(tb2, dtype=np.float32)

    # Fold tw1 into the tables: PT[f] = tables[f] @ tw1_f  -> [F, CARD, 16]
    tw1_e = tw1[: F * D].reshape(F, D, H_TOP)
    pt = np.einsum("fcd,fdh->fch", tables, tw1_e, optimize=True).astype(np.float32)
    # Fold the constant hidden-layer offset into table 0.
    c = (b2 @ tw1[F * D :]) + tb1  # [16]
    pt[0] += c
    ptab = np.ascontiguousarray(pt.reshape(F * CARD, H_TOP).astype(TABLE_NP_DT))

    w21 = np.ascontiguousarray(w2 @ tw1[F * D :])  # [8, 16]

    comb_full = np.asarray(sparse_x, dtype=np.int64) + (
        np.arange(F, dtype=np.int64) * CARD
    )
    comb_full = comb_full.astype(np.int32)  # [B, 26] flat PT row ids

    dense_f = np.asarray(dense_x, dtype=np.float32)

    tpg = 4
    cb = np.zeros((P, tpg * H_TOP + 2), dtype=np.float32)
    cb[:, : tpg * H_TOP] = np.tile(tw2.reshape(-1), tpg)
    cb[:, tpg * H_TOP] = tb2[0]
    cb[:H_BOT, tpg * H_TOP + 1] = np.asarray(b1, np.float32)

    wb = np.zeros((DENSE, H_BOT + H_TOP), dtype=np.float16)
    wb[:, :H_BOT] = np.asarray(w1, np.float32).astype(np.float16)
    wb[:H_BOT, H_BOT : H_BOT + H_TOP] = w21.astype(np.float16)

    shared = {
        "ptab": ptab,
        "wblob": wb,
        "cblob": cb,
        "identh": np.eye(P, dtype=np.float16),
    }
    b_loc = B // N_CORES
    n_t = b_loc // P
    in_maps = []
    for c_ in range(N_CORES):
        lo, hi = c_ * b_loc, (c_ + 1) * b_loc
        comb_c = comb_full[lo:hi].reshape(n_t, P, F).transpose(1, 0, 2)
        m = dict(shared)
        m["comb"] = np.ascontiguousarray(comb_c.reshape(P, n_t * F))
        m["dxt"] = np.ascontiguousarray(dense_f[lo:hi].T.astype(np.float16))
        in_maps.append(m)
    return in_maps


def kernel(**inputs):
    from concourse.bass_utils import run_bass_kernel_spmd

    nc = _get_nc()
    in_maps = make_in_maps(**inputs)
    res = run_bass_kernel_spmd(nc, in_maps, core_ids=list(range(N_CORES)))
    out = np.concatenate([r["y"].reshape(-1) for r in res.results])
    return out.reshape(B, 1).astype(np.float32)


# revision 63
# speedup vs baseline: 1.0505x; 1.0505x over previous
"""DLRM forward (embedding gather + tiny MLPs) as a Bass/Tile kernel on 8 trn2 cores.

Sharding: data-parallel over the batch; each of the 8 cores handles B/8 = 2048
samples end-to-end against a full replica of the (read-only) tables.

Key transformation (host-side, exact): the top-MLP first layer is linear in the
embedding concat, so fold tw1 into the tables once per call:
    PT[f] = tables[f] @ tw1[f*64:(f+1)*64, :]        # [CARD, 16] per table
    hidden[b] = sum_f PT[f][idx[b,f]] + relu(x@w1+b1) @ (w2@tw1_d) + (b2@tw1_d + tb1)
The constant vector (b2@tw1_d + tb1) is folded into PT table 0. The device
then gathers 64B fp32 rows (26 per sample) and reduces them on DVE — no
[B, 26, 64] materialization, no PE transposes of embeddings, and the gather
descriptor stream (53248 64B rows/core) runs at the DMA floor.

Per-core pipeline:
  - 16 indirect DMAs (one per 128-sample tile) gather 26x16 f32 into one big
    SBUF tile; DVE reduces each [128, 26, 16] block over f (axis-X reduce on a
    strided view) into gathersum [128, 16].
  - bottom MLP runs feature-major on PE ([13,2048] loaded pre-transposed from
    host): w1 matmul -> relu -> (w2@tw1_d) matmul -> PE-transpose back to
    sample-major [128, 16] chunks in PSUM.
  - DVE adds gathersum + dense chunk, applies relu*tw2 in one
    scalar_tensor_tensor, reduces over the 16 hidden units; ACT applies
    sigmoid(+tb2); one final PE transpose lays y out [16, 128] for a single
    contiguous output DMA.
"""

import numpy as np

import concourse.bass as bass
import concourse.mybir as mybir
import concourse.tile as tile
from concourse import bacc

P = 128

# Problem constants (hardcoded per harness contract).
N_CORES = 8
B = 16384
F = 26
D = 64
DENSE = 13
CARD = 100000
H_BOT = 8
H_TOP = 16

f32 = mybir.dt.float32
i32 = mybir.dt.int32

GATHER_SIZES = [2, 4, 4, 2, 2, 2]  # tiles of 128 samples per indirect DMA
POOL_GROUP = 3  # group of 4 tiles whose 26-sum + head runs on gpsimd
POOL_FROM = 12  # first tile whose 26-sum runs on gpsimd (rest go to DVE)
TABLE_DT = mybir.dt.float16
TABLE_NP_DT = np.float16
f16 = mybir.dt.float16


def build_kernel(
    b_loc=B // N_CORES,
    card=CARD,
    n_f=F,
    n_dense=DENSE,
    h_bot=H_BOT,
    h_top=H_TOP,
):
    v = n_f * card
    n_t = b_loc // P  # 16 tiles of 128 samples
    group = min(512, b_loc)  # batch columns per matmul group
    tpg = group // P  # tiles per group (4)
    n_g = b_loc // group  # groups (4)
    row = h_top  # gathered row length (16 f32)
    trow = n_f * row  # per-tile gather width (416)

    nc = bacc.Bacc("TRN2", target_bir_lowering=False)
    ptab_d = nc.dram_tensor("ptab", [v, row], TABLE_DT, kind="ExternalInput")
    comb_d = nc.dram_tensor("comb", [P, n_t * n_f], i32, kind="ExternalInput")
    dxt_d = nc.dram_tensor("dxt", [n_dense, b_loc], f16, kind="ExternalInput")
    # wblob[:, 0:8] = w1 (13 rows); wblob[0:8, 8:24] = w2@tw1_d  (fp16)
    wb_d = nc.dram_tensor("wblob", [n_dense, h_bot + h_top], f16, kind="ExternalInput")
    # cblob[:, 0:64] = tw2 tiled 4x; cblob[:, 64] = tb2; cblob[0:8, 65] = b1
    cb_d = nc.dram_tensor("cblob", [P, tpg * h_top + 2], f32, kind="ExternalInput")
    id_d = nc.dram_tensor("identh", [P, P], f16, kind="ExternalInput")
    # sample-major output: y[p, t] = sample t*128+p; host transposes (free)
    y_d = nc.dram_tensor("y", [P, n_t], f32, kind="ExternalOutput")

    with tile.TileContext(nc) as tc:
        with (
            tc.tile_pool(name="const", bufs=1) as cpool,
            tc.tile_pool(name="small", bufs=2) as smallp,
            tc.tile_pool(name="pmm", bufs=2, space="PSUM") as pmmp,
            tc.tile_pool(name="pd", bufs=2, space="PSUM") as pdp,
            tc.tile_pool(name="pfix", bufs=1, space="PSUM") as pfixp,
        ):
            # ---- dummy sigmoid: forces the one ACT table set that contains
            # sigmoid+relu+copy to load now, so no reload sits on the tail ----
            scr0 = cpool.tile([1, 1], f32)
            nc.gpsimd.memset(scr0[:], 0.0)
            scr1 = cpool.tile([1, 1], f32)
            nc.scalar.activation(
                out=scr1[:],
                in_=scr0[:],
                func=mybir.ActivationFunctionType.Sigmoid,
            )

            # ---- index upload: sole occupant of the sync (SP) queue, so the
            # first gather waits on exactly one DMA; split so the first
            # gather's slice lands first ----
            c0 = GATHER_SIZES[0] * n_f
            comb = cpool.tile([P, n_t * n_f], i32)
            nc.sync.dma_start(out=comb[:, 0:c0], in_=comb_d[:, 0:c0])
            nc.sync.dma_start(out=comb[:, c0 : n_t * n_f], in_=comb_d[:, c0 : n_t * n_f])

            # ---- constants via the Activation HWDGE queue (independent) ----
            dxt = cpool.tile([n_dense, b_loc], f16)
            nc.scalar.dma_start(out=dxt[:], in_=dxt_d[:, :])
            wb = cpool.tile([n_dense, h_bot + h_top], f16)
            nc.scalar.dma_start(out=wb[:], in_=wb_d[:, :])
            w1_sb = wb[:, 0:h_bot]
            w21_sb = wb[0:h_bot, h_bot : h_bot + h_top]
            cb = cpool.tile([P, tpg * h_top + 2], f32)
            nc.scalar.dma_start(out=cb[:], in_=cb_d[:, :])
            tw2b = cb[:, 0 : tpg * h_top]
            tb2b = cb[:, tpg * h_top : tpg * h_top + 1]
            b1_sb = cb[0:h_bot, tpg * h_top + 1 : tpg * h_top + 2]
            ident = cpool.tile([P, P], f16)
            nc.scalar.dma_start(out=ident[:], in_=id_d[:, :])

            # one SBUF tile per gather chunk: a single big tile would make
            # every reader wait on ALL gathers (subtile dep tracking lumps
            # the regions), serializing the whole reduce pipeline
            et_tiles = {}
            t0 = 0
            for sz in GATHER_SIZES:
                et = cpool.tile([P, sz * trow], TABLE_DT, name=f"et_{t0}")
                for t in range(t0, t0 + sz):
                    et_tiles[t] = (et, t - t0)
                t0 += sz
            gs = cpool.tile([P, n_t * h_top], f32)  # gathersum [128, 256]
            hs = cpool.tile([P, n_t * h_top], f32)  # hidden pre-relu
            mm = cpool.tile([P, n_t * h_top], f32)  # relu(h) * tw2
            lg = cpool.tile([P, n_t], f32)  # logits [128, 16]
            ylog = cpool.tile([P, n_t], f32)
            pdhT = pfixp.tile([P, n_t * h_top], f16)  # dense hidden, sample-major
            # the gpsimd-owned group gets its own PSUM tile: sharing one tile
            # with the DVE heads makes the ACT staging copy inherit their
            # (slice-unrelated) deps and stalls it by ~8us
            pdhT2 = pfixp.tile([P, tpg * h_top], f16)

            # ---- bottom MLP, feature-major, wave-scheduled so the in-order
            # PE queue never stalls on a not-yet-ready cross-engine dep ----
            phs, h1s, pds, dhs = [], [], [], []
            for g in range(n_g):
                ph = pmmp.tile([h_bot, group], f32, tag="pmm")
                nc.tensor.matmul(
                    out=ph[:],
                    lhsT=w1_sb,
                    rhs=dxt[:, bass.ts(g, group)],
                    start=True,
                    stop=True,
                )
                phs.append(ph)
            for g in range(n_g):
                h1 = smallp.tile([h_bot, group], f16, tag="h1")
                nc.scalar.activation(
                    out=h1[:],
                    in_=phs[g][:],
                    func=mybir.ActivationFunctionType.Relu,
                    bias=b1_sb,
                )
                h1s.append(h1)
            for g in range(n_g):
                pd = pdp.tile([h_top, group], f32, tag="pd")
                nc.tensor.matmul(
                    out=pd[:], lhsT=w21_sb, rhs=h1s[g][:], start=True, stop=True
                )
                pds.append(pd)
            for g in range(n_g):
                dh = smallp.tile([h_top, group], f16, tag="dh")
                nc.scalar.activation(
                    out=dh[:],
                    in_=pds[g][:],
                    func=mybir.ActivationFunctionType.Copy,
                )
                dhs.append(dh)
            for g in range(n_g):
                for j in range(tpg):
                    dst = (
                        pdhT2[:, bass.ts(j, h_top)]
                        if g == POOL_GROUP
                        else pdhT[:, bass.ts(g * tpg + j, h_top)]
                    )
                    nc.tensor.transpose(
                        out=dst,
                        in_=dhs[g][:, bass.ts(j, P)],
                        identity=ident[0:h_top, 0:h_top],
                    )

            # ---- gathers, one strided reduce chasing each gather,
            # per-group head ops interleaved ----
            def emit_head(g):
                gcols = bass.ts(g, tpg * h_top)
                nc.vector.tensor_tensor(
                    out=hs[:, gcols],
                    in0=gs[:, gcols],
                    in1=pdhT[:, gcols],
                    op=mybir.AluOpType.add,
                )
                # relu then scale by tw2 (broadcast across partitions)
                nc.vector.scalar_tensor_tensor(
                    out=mm[:, gcols],
                    in0=hs[:, gcols],
                    scalar=0.0,
                    in1=tw2b,
                    op0=mybir.AluOpType.max,
                    op1=mybir.AluOpType.mult,
                )
                nc.vector.tensor_reduce(
                    out=lg[:, bass.ts(g, tpg)],
                    in_=mm[:, gcols].rearrange("p (t j) -> p t j", t=tpg),
                    axis=mybir.AxisListType.X,
                    op=mybir.AluOpType.add,
                )

            p_lo, p_hi = POOL_GROUP * tpg, (POOL_GROUP + 1) * tpg
            t0 = 0
            for gi, sz in enumerate(GATHER_SIZES):
                t1 = min(t0 + sz, n_t)
                et = et_tiles[t0][0]
                nc.gpsimd.indirect_dma_start(
                    out=et[:],
                    out_offset=None,
                    in_=ptab_d[:, :],
                    in_offset=bass.IndirectOffsetOnAxis(
                        ap=comb[:, t0 * n_f : t1 * n_f], axis=0
                    ),
                )
                # sum over the 26 tables, one DVE reduce per 128-sample tile
                for t in range(t0, t1):
                    if t >= POOL_FROM:
                        continue  # gpsimd-owned
                    nc.vector.tensor_reduce(
                        out=gs[:, bass.ts(t, h_top)],
                        in_=et[:, bass.ts(t - t0, trow)].rearrange(
                            "p (f j) -> p j f", f=n_f
                        ),
                        axis=mybir.AxisListType.X,
                        op=mybir.AluOpType.add,
                    )
                    if t % tpg == tpg - 1:
                        emit_head(t // tpg)
                t0 = t1

            # ---- gpsimd-owned group: 26-sum tree + head add on gpsimd (idle
            # after desc gen), in parallel with DVE's tiles ----
            if POOL_GROUP < n_g:
                t = POOL_FROM
                ci = 0
                while t < n_t:
                    et, off = et_tiles[t]
                    assert off == 0, "pool chunks must align to gather tiles"
                    K = et.shape[1] // trow
                    assert t + K <= n_t
                    va = et[:].rearrange("p (t f j) -> p t f j", t=K, f=n_f)
                    scrA = cpool.tile([P, K * 13 * h_top], f16, name=f"scrA{ci}")
                    vA = scrA[:].rearrange("p (t f j) -> p t f j", t=K, f=13)
                    nc.gpsimd.tensor_tensor(
                        out=vA, in0=va[:, :, 0:13], in1=va[:, :, 13:26],
                        op=mybir.AluOpType.add,
                    )
                    scrB = cpool.tile([P, K * 6 * h_top], f16, name=f"scrB{ci}")
                    vB = scrB[:].rearrange("p (t f j) -> p t f j", t=K, f=6)
                    nc.gpsimd.tensor_tensor(
                        out=vB, in0=vA[:, :, 0:6], in1=vA[:, :, 6:12],
                        op=mybir.AluOpType.add,
                    )
                    scrC = cpool.tile([P, K * 3 * h_top], f16, name=f"scrC{ci}")
                    vC = scrC[:].rearrange("p (t f j) -> p t f j", t=K, f=3)
                    nc.gpsimd.tensor_tensor(
                        out=vC, in0=vB[:, :, 0:3], in1=vB[:, :, 3:6],
                        op=mybir.AluOpType.add,
                    )
                    scrD = cpool.tile([P, K * h_top], f16, name=f"scrD{ci}")
                    vD = scrD[:].rearrange("p (t c j) -> p t c j", t=K, c=1)
                    nc.gpsimd.tensor_tensor(
                        out=vD, in0=vC[:, :, 0:1], in1=vC[:, :, 1:2],
                        op=mybir.AluOpType.add,
                    )
                    nc.gpsimd.tensor_tensor(
                        out=vD, in0=vD, in1=vC[:, :, 2:3], op=mybir.AluOpType.add
                    )
                    gsl = gs[:, t * h_top : (t + K) * h_top].rearrange(
                        "p (t c j) -> p t c j", t=K, c=1
                    )
                    nc.gpsimd.tensor_tensor(
                        out=gsl, in0=vD, in1=vA[:, :, 12:13], op=mybir.AluOpType.add
                    )
                    t += K
                    ci += 1
                # head for the gpsimd-owned group;
                # gpsimd cannot read PSUM, so stage dense-hidden via ACT
                dhT_sb = cpool.tile([P, tpg * h_top], f32)
                nc.scalar.activation(
                    out=dhT_sb[:],
                    in_=pdhT2[:],
                    func=mybir.ActivationFunctionType.Copy,
                )
                hsl = hs[:, p_lo * h_top : p_hi * h_top]
                nc.gpsimd.tensor_tensor(
                    out=hsl,
                    in0=gs[:, p_lo * h_top : p_hi * h_top],
                    in1=dhT_sb[:],
                    op=mybir.AluOpType.add,
                )
                # heads for groups whose gs is (partly) gpsimd-written but
                # that read pdhT from PSUM: run on DVE at the end of its queue
                for g_mid in range(POOL_FROM // tpg, POOL_GROUP):
                    emit_head(g_mid)
                # relu*tw2 + reduce for these tiles runs on DVE (appended
                # after its own queue; TensorScalarPtr is not a Pool opcode)
                g_last = POOL_GROUP
                gcols = bass.ts(g_last, tpg * h_top)
                nc.vector.scalar_tensor_tensor(
                    out=mm[:, gcols],
                    in0=hs[:, gcols],
                    scalar=0.0,
                    in1=tw2b,
                    op0=mybir.AluOpType.max,
                    op1=mybir.AluOpType.mult,
                )
                nc.vector.tensor_reduce(
                    out=lg[:, bass.ts(g_last, tpg)],
                    in_=mm[:, gcols].rearrange("p (t j) -> p t j", t=tpg),
                    axis=mybir.AxisListType.X,
                    op=mybir.AluOpType.add,
                )

            nc.scalar.activation(
                out=ylog[:],
                in_=lg[:],
                func=mybir.ActivationFunctionType.Sigmoid,
                bias=tb2b,
            )
            nc.sync.dma_start(out=y_d[:, :], in_=ylog[:])

    nc.compile()
    return nc


_NC_CACHE = {}


def _get_nc():
    if "nc" not in _NC_CACHE:
        _NC_CACHE["nc"] = build_kernel()
    return _NC_CACHE["nc"]


def make_in_maps(dense_x, sparse_x, tables, w1, b1, w2, b2, tw1, tb1, tw2, tb2):
    tables = np.asarray(tables, dtype=np.float32)
    tw1 = np.asarray(tw1, dtype=np.float32)
    tw2 = np.asarray(tw2, dtype=np.float32)
    w2 = np.asarray(w2, dtype=np.float32)
    b2 = np.asarray(b2, dtype=np.float32)
    tb1 = np.asarray(tb1, dtype=np.float32)
    tb2 = np.asarray(tb2, dtype=np.float32)

    # Fold tw1 into the tables: PT[f] = tables[f] @ tw1_f  -> [F, CARD, 16]
    tw1_e = tw1[: F * D].reshape(F, D, H_TOP)
    pt = np.einsum("fcd,fdh->fch", tables, tw1_e, optimize=True).astype(np.float32)
    # Fold the constant hidden-layer offset into table 0.
    c = (b2 @ tw1[F * D :]) + tb1  # [16]
    pt[0] += c
    ptab = np.ascontiguousarray(pt.reshape(F * CARD, H_TOP).astype(TABLE_NP_DT))

    w21 = np.ascontiguousarray(w2 @ tw1[F * D :])  # [8, 16]

    comb_full = np.asarray(sparse_x, dtype=np.int64) + (
        np.arange(F, dtype=np.int64) * CARD
    )
    comb_full = comb_full.astype(np.int32)  # [B, 26] flat PT row ids

    dense_f = np.asarray(dense_x, dtype=np.float32)

    tpg = 4
    cb = np.zeros((P, tpg * H_TOP + 2), dtype=np.float32)
    cb[:, : tpg * H_TOP] = np.tile(tw2.reshape(-1), tpg)
    cb[:, tpg * H_TOP] = tb2[0]
    cb[:H_BOT, tpg * H_TOP + 1] = np.asarray(b1, np.float32)

    wb = np.zeros((DENSE, H_BOT + H_TOP), dtype=np.float16)
    wb[:, :H_BOT] = np.asarray(w1, np.float32).astype(np.float16)
    wb[:H_BOT, H_BOT : H_BOT + H_TOP] = w21.astype(np.float16)

    shared = {
        "ptab": ptab,
        "wblob": wb,
        "cblob": cb,
        "identh": np.eye(P, dtype=np.float16),
    }
    b_loc = B // N_CORES
    n_t = b_loc // P
    in_maps = []
    for c_ in range(N_CORES):
        lo, hi = c_ * b_loc, (c_ + 1) * b_loc
        comb_c = comb_full[lo:hi].reshape(n_t, P, F).transpose(1, 0, 2)
        m = dict(shared)
        m["comb"] = np.ascontiguousarray(comb_c.reshape(P, n_t * F))
        m["dxt"] = np.ascontiguousarray(dense_f[lo:hi].T.astype(np.float16))
        in_maps.append(m)
    return in_maps


def kernel(**inputs):
    from concourse.bass_utils import run_bass_kernel_spmd

    nc = _get_nc()
    in_maps = make_in_maps(**inputs)
    res = run_bass_kernel_spmd(nc, in_maps, core_ids=list(range(N_CORES)))
    # device y is [128 partitions, 16 tiles]; sample t*128+p lives at [p, t]
    out = np.concatenate([r["y"].T.reshape(-1) for r in res.results])
    return out.reshape(B, 1).astype(np.float32)


# revision 64
# speedup vs baseline: 1.1113x; 1.0579x over previous
"""DLRM forward (embedding gather + tiny MLPs) as a Bass/Tile kernel on 8 trn2 cores.

Sharding: data-parallel over the batch; each of the 8 cores handles B/8 = 2048
samples end-to-end against a full replica of the (read-only) tables.

Key transformation (host-side, exact): the top-MLP first layer is linear in the
embedding concat, so fold tw1 into the tables once per call:
    PT[f] = tables[f] @ tw1[f*64:(f+1)*64, :]        # [CARD, 16] per table
    hidden[b] = sum_f PT[f][idx[b,f]] + relu(x@w1+b1) @ (w2@tw1_d) + (b2@tw1_d + tb1)
The constant vector (b2@tw1_d + tb1) is folded into PT table 0. The device
then gathers 64B fp32 rows (26 per sample) and reduces them on DVE — no
[B, 26, 64] materialization, no PE transposes of embeddings, and the gather
descriptor stream (53248 64B rows/core) runs at the DMA floor.

Per-core pipeline:
  - 16 indirect DMAs (one per 128-sample tile) gather 26x16 f32 into one big
    SBUF tile; DVE reduces each [128, 26, 16] block over f (axis-X reduce on a
    strided view) into gathersum [128, 16].
  - bottom MLP runs feature-major on PE ([13,2048] loaded pre-transposed from
    host): w1 matmul -> relu -> (w2@tw1_d) matmul -> PE-transpose back to
    sample-major [128, 16] chunks in PSUM.
  - DVE adds gathersum + dense chunk, applies relu*tw2 in one
    scalar_tensor_tensor, reduces over the 16 hidden units; ACT applies
    sigmoid(+tb2); one final PE transpose lays y out [16, 128] for a single
    contiguous output DMA.
"""

import numpy as np

import concourse.bass as bass
import concourse.mybir as mybir
import concourse.tile as tile
from concourse import bacc

P = 128

# Problem constants (hardcoded per harness contract).
N_CORES = 8
B = 16384
F = 26
D = 64
DENSE = 13
CARD = 100000
H_BOT = 8
H_TOP = 16

f32 = mybir.dt.float32
i32 = mybir.dt.int32

GATHER_SIZES = [2, 4, 4, 2, 2, 2]  # tiles of 128 samples per indirect DMA
POOL_GROUP = 3  # group of 4 tiles whose 26-sum + head runs on gpsimd
POOL_FROM = 12  # first tile whose 26-sum runs on gpsimd (rest go to DVE)
TABLE_DT = mybir.dt.float16
TABLE_NP_DT = np.float16
f16 = mybir.dt.float16


def build_kernel(
    b_loc=B // N_CORES,
    card=CARD,
    n_f=F,
    n_dense=DENSE,
    h_bot=H_BOT,
    h_top=H_TOP,
):
    v = n_f * card
    n_t = b_loc // P  # 16 tiles of 128 samples
    group = min(512, b_loc)  # batch columns per matmul group
    tpg = group // P  # tiles per group (4)
    n_g = b_loc // group  # groups (4)
    row = h_top  # gathered row length (16 f32)
    trow = n_f * row  # per-tile gather width (416)

    nc = bacc.Bacc("TRN2", target_bir_lowering=False)
    ptab_d = nc.dram_tensor("ptab", [v, row], TABLE_DT, kind="ExternalInput")
    comb_d = nc.dram_tensor("comb", [P, n_t * n_f], i32, kind="ExternalInput")
    dxt_d = nc.dram_tensor("dxt", [n_dense, b_loc], f16, kind="ExternalInput")
    # wblob[:, 0:8] = w1 (13 rows); wblob[0:8, 8:24] = w2@tw1_d  (fp16)
    wb_d = nc.dram_tensor("wblob", [n_dense, h_bot + h_top], f16, kind="ExternalInput")
    # cblob[:, 0:64] = tw2 tiled 4x; cblob[:, 64] = tb2; cblob[0:8, 65] = b1
    cb_d = nc.dram_tensor("cblob", [P, tpg * h_top + 2], f32, kind="ExternalInput")
    id_d = nc.dram_tensor("identh", [P, P], f16, kind="ExternalInput")
    # sample-major output: y[p, t] = sample t*128+p; host transposes (free)
    y_d = nc.dram_tensor("y", [P, n_t], f32, kind="ExternalOutput")

    with tile.TileContext(nc) as tc:
        with (
            tc.tile_pool(name="const", bufs=1) as cpool,
            tc.tile_pool(name="small", bufs=2) as smallp,
            tc.tile_pool(name="pmm", bufs=2, space="PSUM") as pmmp,
            tc.tile_pool(name="pd", bufs=2, space="PSUM") as pdp,
            tc.tile_pool(name="pfix", bufs=1, space="PSUM") as pfixp,
        ):
            # ---- dummy sigmoid: forces the one ACT table set that contains
            # sigmoid+relu+copy to load now, so no reload sits on the tail ----
            scr0 = cpool.tile([1, 1], f32)
            nc.gpsimd.memset(scr0[:], 0.0)
            scr1 = cpool.tile([1, 1], f32)
            nc.scalar.activation(
                out=scr1[:],
                in_=scr0[:],
                func=mybir.ActivationFunctionType.Sigmoid,
            )

            # ---- index upload: sole occupant of the sync (SP) queue, so the
            # first gather waits on exactly one DMA; split so the first
            # gather's slice lands first ----
            c0 = GATHER_SIZES[0] * n_f
            comb = cpool.tile([P, n_t * n_f], i32)
            nc.sync.dma_start(out=comb[:, 0:c0], in_=comb_d[:, 0:c0])
            nc.sync.dma_start(out=comb[:, c0 : n_t * n_f], in_=comb_d[:, c0 : n_t * n_f])

            # ---- constants via the Activation HWDGE queue (independent) ----
            dxt = cpool.tile([n_dense, b_loc], f16)
            nc.scalar.dma_start(out=dxt[:], in_=dxt_d[:, :])
            wb = cpool.tile([n_dense, h_bot + h_top], f16)
            nc.scalar.dma_start(out=wb[:], in_=wb_d[:, :])
            w1_sb = wb[:, 0:h_bot]
            w21_sb = wb[0:h_bot, h_bot : h_bot + h_top]
            cb = cpool.tile([P, tpg * h_top + 2], f32)
            nc.scalar.dma_start(out=cb[:], in_=cb_d[:, :])
            tw2b = cb[:, 0 : tpg * h_top]
            tb2b = cb[:, tpg * h_top : tpg * h_top + 1]
            b1_sb = cb[0:h_bot, tpg * h_top + 1 : tpg * h_top + 2]
            ident = cpool.tile([P, P], f16)
            nc.scalar.dma_start(out=ident[:], in_=id_d[:, :])

            # one SBUF tile per gather chunk: a single big tile would make
            # every reader wait on ALL gathers (subtile dep tracking lumps
            # the regions), serializing the whole reduce pipeline
            et_tiles = {}
            t0 = 0
            for sz in GATHER_SIZES:
                et = cpool.tile([P, sz * trow], TABLE_DT, name=f"et_{t0}")
                for t in range(t0, t0 + sz):
                    et_tiles[t] = (et, t - t0)
                t0 += sz
            gs = cpool.tile([P, n_t * h_top], f32)  # gathersum [128, 256]
            hs = cpool.tile([P, n_t * h_top], f32)  # hidden pre-relu
            mm = cpool.tile([P, n_t * h_top], f32)  # relu(h) * tw2
            lg = cpool.tile([P, n_t], f32)  # logits [128, 16]
            ylog = cpool.tile([P, n_t], f32)
            pdhT = pfixp.tile([P, n_t * h_top], f16)  # dense hidden, sample-major
            # the gpsimd-owned group gets its own PSUM tile: sharing one tile
            # with the DVE heads makes the ACT staging copy inherit their
            # (slice-unrelated) deps and stalls it by ~8us
            pdhT2 = pfixp.tile([P, tpg * h_top], f16)

            # ---- bottom MLP, feature-major, wave-scheduled so the in-order
            # PE queue never stalls on a not-yet-ready cross-engine dep ----
            phs, h1s, pds, dhs = [], [], [], []
            for g in range(n_g):
                ph = pmmp.tile([h_bot, group], f32, tag="pmm")
                nc.tensor.matmul(
                    out=ph[:],
                    lhsT=w1_sb,
                    rhs=dxt[:, bass.ts(g, group)],
                    start=True,
                    stop=True,
                )
                phs.append(ph)
            for g in range(n_g):
                h1 = smallp.tile([h_bot, group], f16, tag="h1")
                nc.scalar.activation(
                    out=h1[:],
                    in_=phs[g][:],
                    func=mybir.ActivationFunctionType.Relu,
                    bias=b1_sb,
                )
                h1s.append(h1)
            for g in range(n_g):
                pd = pdp.tile([h_top, group], f32, tag="pd")
                nc.tensor.matmul(
                    out=pd[:], lhsT=w21_sb, rhs=h1s[g][:], start=True, stop=True
                )
                pds.append(pd)
            for g in range(n_g):
                dh = smallp.tile([h_top, group], f16, tag="dh")
                nc.scalar.activation(
                    out=dh[:],
                    in_=pds[g][:],
                    func=mybir.ActivationFunctionType.Copy,
                )
                dhs.append(dh)
            for g in range(n_g):
                for j in range(tpg):
                    dst = (
                        pdhT2[:, bass.ts(j, h_top)]
                        if g == POOL_GROUP
                        else pdhT[:, bass.ts(g * tpg + j, h_top)]
                    )
                    nc.tensor.transpose(
                        out=dst,
                        in_=dhs[g][:, bass.ts(j, P)],
                        identity=ident[0:h_top, 0:h_top],
                    )

            # ---- gathers, one strided reduce chasing each gather,
            # per-group head ops interleaved ----
            def emit_head(g):
                gcols = bass.ts(g, tpg * h_top)
                nc.vector.tensor_tensor(
                    out=hs[:, gcols],
                    in0=gs[:, gcols],
                    in1=pdhT[:, gcols],
                    op=mybir.AluOpType.add,
                )
                # relu then scale by tw2 (broadcast across partitions)
                nc.vector.scalar_tensor_tensor(
                    out=mm[:, gcols],
                    in0=hs[:, gcols],
                    scalar=0.0,
                    in1=tw2b,
                    op0=mybir.AluOpType.max,
                    op1=mybir.AluOpType.mult,
                )
                nc.vector.tensor_reduce(
                    out=lg[:, bass.ts(g, tpg)],
                    in_=mm[:, gcols].rearrange("p (t j) -> p t j", t=tpg),
                    axis=mybir.AxisListType.X,
                    op=mybir.AluOpType.add,
                )

            p_lo, p_hi = POOL_GROUP * tpg, (POOL_GROUP + 1) * tpg
            t0 = 0
            for gi, sz in enumerate(GATHER_SIZES):
                t1 = min(t0 + sz, n_t)
                et = et_tiles[t0][0]
                nc.gpsimd.indirect_dma_start(
                    out=et[:],
                    out_offset=None,
                    in_=ptab_d[:, :],
                    in_offset=bass.IndirectOffsetOnAxis(
                        ap=comb[:, t0 * n_f : t1 * n_f], axis=0
                    ),
                )
                # sum over the 26 tables, one DVE reduce per 128-sample tile
                for t in range(t0, t1):
                    if t >= POOL_FROM:
                        continue  # gpsimd-owned
                    nc.vector.tensor_reduce(
                        out=gs[:, bass.ts(t, h_top)],
                        in_=et[:, bass.ts(t - t0, trow)].rearrange(
                            "p (f j) -> p j f", f=n_f
                        ),
                        axis=mybir.AxisListType.X,
                        op=mybir.AluOpType.add,
                    )
                    if t % tpg == tpg - 1:
                        emit_head(t // tpg)
                t0 = t1

            # ---- gpsimd-owned group: 26-sum tree + head add on gpsimd (idle
            # after desc gen), in parallel with DVE's tiles ----
            if POOL_GROUP < n_g:
                t = POOL_FROM
                ci = 0
                while t < n_t:
                    et, off = et_tiles[t]
                    assert off == 0, "pool chunks must align to gather tiles"
                    K = et.shape[1] // trow
                    assert t + K <= n_t
                    va = et[:].rearrange("p (t f j) -> p t f j", t=K, f=n_f)
                    scrA = cpool.tile([P, K * 13 * h_top], f16, name=f"scrA{ci}")
                    vA = scrA[:].rearrange("p (t f j) -> p t f j", t=K, f=13)
                    nc.gpsimd.tensor_tensor(
                        out=vA, in0=va[:, :, 0:13], in1=va[:, :, 13:26],
                        op=mybir.AluOpType.add,
                    )
                    scrB = cpool.tile([P, K * 6 * h_top], f16, name=f"scrB{ci}")
                    vB = scrB[:].rearrange("p (t f j) -> p t f j", t=K, f=6)
                    nc.gpsimd.tensor_tensor(
                        out=vB, in0=vA[:, :, 0:6], in1=vA[:, :, 6:12],
                        op=mybir.AluOpType.add,
                    )
                    scrC = cpool.tile([P, K * 3 * h_top], f16, name=f"scrC{ci}")
                    vC = scrC[:].rearrange("p (t f j) -> p t f j", t=K, f=3)
                    nc.gpsimd.tensor_tensor(
                        out=vC, in0=vB[:, :, 0:3], in1=vB[:, :, 3:6],
                        op=mybir.AluOpType.add,
                    )
                    scrD = cpool.tile([P, K * h_top], f16, name=f"scrD{ci}")
                    vD = scrD[:].rearrange("p (t c j) -> p t c j", t=K, c=1)
                    nc.gpsimd.tensor_tensor(
                        out=vD, in0=vC[:, :, 0:1], in1=vC[:, :, 1:2],
                        op=mybir.AluOpType.add,
                    )
                    nc.gpsimd.tensor_tensor(
                        out=vD, in0=vD, in1=vC[:, :, 2:3], op=mybir.AluOpType.add
                    )
                    gsl = gs[:, t * h_top : (t + K) * h_top].rearrange(
                        "p (t c j) -> p t c j", t=K, c=1
                    )
                    nc.gpsimd.tensor_tensor(
                        out=gsl, in0=vD, in1=vA[:, :, 12:13], op=mybir.AluOpType.add
                    )
                    t += K
                    ci += 1
                # heads for groups whose gs is (partly) gpsimd-written but
                # that read pdhT from PSUM: run on DVE at the end of its queue
                for g_mid in range(POOL_FROM // tpg, POOL_GROUP):
                    emit_head(g_mid)
                # head for the gpsimd-owned group: fully on DVE (idle by now;
                # reads the trees' gs via one cross sem and pdhT2 from PSUM
                # directly — no ACT staging copy, no gpsimd add)
                g_last = POOL_GROUP
                gcols = bass.ts(g_last, tpg * h_top)
                nc.vector.tensor_tensor(
                    out=hs[:, gcols],
                    in0=gs[:, gcols],
                    in1=pdhT2[:],
                    op=mybir.AluOpType.add,
                )
                nc.vector.scalar_tensor_tensor(
                    out=mm[:, gcols],
                    in0=hs[:, gcols],
                    scalar=0.0,
                    in1=tw2b,
                    op0=mybir.AluOpType.max,
                    op1=mybir.AluOpType.mult,
                )
                nc.vector.tensor_reduce(
                    out=lg[:, bass.ts(g_last, tpg)],
                    in_=mm[:, gcols].rearrange("p (t j) -> p t j", t=tpg),
                    axis=mybir.AxisListType.X,
                    op=mybir.AluOpType.add,
                )

            nc.scalar.activation(
                out=ylog[:],
                in_=lg[:],
                func=mybir.ActivationFunctionType.Sigmoid,
                bias=tb2b,
            )
            nc.sync.dma_start(out=y_d[:, :], in_=ylog[:])

    nc.compile()
    return nc


_NC_CACHE = {}


def _get_nc():
    if "nc" not in _NC_CACHE:
        _NC_CACHE["nc"] = build_kernel()
    return _NC_CACHE["nc"]


def make_in_maps(dense_x, sparse_x, tables, w1, b1, w2, b2, tw1, tb1, tw2, tb2):
    tables = np.asarray(tables, dtype=np.float32)
    tw1 = np.asarray(tw1, dtype=np.float32)
    tw2 = np.asarray(tw2, dtype=np.float32)
    w2 = np.asarray(w2, dtype=np.float32)
    b2 = np.asarray(b2, dtype=np.float32)
    tb1 = np.asarray(tb1, dtype=np.float32)
    tb2 = np.asarray(tb2, dtype=np.float32)

    # Fold tw1 into the tables: PT[f] = tables[f] @ tw1_f  -> [F, CARD, 16]
    tw1_e = tw1[: F * D].reshape(F, D, H_TOP)
    pt = np.einsum("fcd,fdh->fch", tables, tw1_e, optimize=True).astype(np.float32)
    # Fold the constant hidden-layer offset into table 0.
    c = (b2 @ tw1[F * D :]) + tb1  # [16]
    pt[0] += c
    ptab = np.ascontiguousarray(pt.reshape(F * CARD, H_TOP).astype(TABLE_NP_DT))

    w21 = np.ascontiguousarray(w2 @ tw1[F * D :])  # [8, 16]

    comb_full = np.asarray(sparse_x, dtype=np.int64) + (
        np.arange(F, dtype=np.int64) * CARD
    )
    comb_full = comb_full.astype(np.int32)  # [B, 26] flat PT row ids

    dense_f = np.asarray(dense_x, dtype=np.float32)

    tpg = 4
    cb = np.zeros((P, tpg * H_TOP + 2), dtype=np.float32)
    cb[:, : tpg * H_TOP] = np.tile(tw2.reshape(-1), tpg)
    cb[:, tpg * H_TOP] = tb2[0]
    cb[:H_BOT, tpg * H_TOP + 1] = np.asarray(b1, np.float32)

    wb = np.zeros((DENSE, H_BOT + H_TOP), dtype=np.float16)
    wb[:, :H_BOT] = np.asarray(w1, np.float32).astype(np.float16)
    wb[:H_BOT, H_BOT : H_BOT + H_TOP] = w21.astype(np.float16)

    shared = {
        "ptab": ptab,
        "wblob": wb,
        "cblob": cb,
        "identh": np.eye(P, dtype=np.float16),
    }
    b_loc = B // N_CORES
    n_t = b_loc // P
    in_maps = []
    for c_ in range(N_CORES):
        lo, hi = c_ * b_loc, (c_ + 1) * b_loc
        comb_c = comb_full[lo:hi].reshape(n_t, P, F).transpose(1, 0, 2)
        m = dict(shared)
        m["comb"] = np.ascontiguousarray(comb_c.reshape(P, n_t * F))
        m["dxt"] = np.ascontiguousarray(dense_f[lo:hi].T.astype(np.float16))
        in_maps.append(m)
    return in_maps


def kernel(**inputs):
    from concourse.bass_utils import run_bass_kernel_spmd

    nc = _get_nc()
    in_maps = make_in_maps(**inputs)
    res = run_bass_kernel_spmd(nc, in_maps, core_ids=list(range(N_CORES)))
    # device y is [128 partitions, 16 tiles]; sample t*128+p lives at [p, t]
    out = np.concatenate([r["y"].T.reshape(-1) for r in res.results])
    return out.reshape(B, 1).astype(np.float32)
